# revision 5
# baseline (speedup 1.0000x reference)
"""GNN autoencoder (2x SAGEConv(mean) -> global mean pool -> MLP head) on 8 trn2 NeuronCores.

Sharding: nodes partitioned contiguously across 8 cores (6250 each, padded to
6272 = 49*128). Edges partitioned by destination core. Per-layer aggregation:
  - compute per-node linear table (xl = x@W1l resp. h1l = h1@W2l) sharded,
    AllGather the table into every core's DRAM,
  - dma_gather the per-edge source rows (sorted by destination block),
  - segment-mean via one-hot "selection" matmuls accumulated in PSUM
    (selection matrix built on-chip from iota==dst_rel, scaled by 1/deg),
  - dense self/bias terms accumulated into the same PSUM group.
Graph pooling via per-block one-hot matmuls + AllReduce; MLP head replicated.
"""

import os
import numpy as np

# ---------------- problem constants (match reference.py) ----------------
N_NODES = 50000
F_IN = 512
N_GRAPHS = 64
F1, F2, LAT, FOUT = 128, 32, 16, 50
NCORES = 8
BLK = 128
CHB = 4  # blocks per gather chunk

_PROG_CACHE = {}


def _cfg(n_nodes, f_in):
    npc = n_nodes // NCORES
    nblk = -(-npc // BLK)
    npad = nblk * BLK
    half = (NCORES // 2) * npad
    kin = f_in // 128
    assert f_in % 128 == 0 and n_nodes % NCORES == 0
    assert half + npad * (NCORES // 2) < 65536 // 2 * 2  # int16 rel indices fit
    return npc, nblk, npad, half, kin


def build_program(n_nodes, f_in, t_blk):
    """Build + compile the SPMD bass program. Returns compiled Bacc."""
    import concourse.bacc as bacc
    import concourse.mybir as mybir
    import concourse.tile as tile
    from concourse.masks import make_identity
    from concourse.tile_rust import add_dep_helper

    npc, nblk, npad, half, kin = _cfg(n_nodes, f_in)
    nidxh = nblk * t_blk * BLK  # indices per half
    f32 = mybir.dt.float32
    i16 = mybir.dt.int16
    AF = mybir.ActivationFunctionType
    ALU = mybir.AluOpType
    rg = [list(range(NCORES))]

    nc = bacc.Bacc("TRN2", num_devices=NCORES, debug=False)

    # ---------------- DRAM tensors ----------------
    xt_d = nc.dram_tensor("xt", [f_in, npad], f32, kind="ExternalInput")
    idx_d = [nc.dram_tensor(f"idx{h}", [128, nidxh // 16], i16, kind="ExternalInput") for h in (0, 1)]
    dstrel_d = nc.dram_tensor("dstrel", [128, 2 * nblk * t_blk], f32, kind="ExternalInput")
    invc_d = nc.dram_tensor("invc", [128, 2 * nblk * t_blk], f32, kind="ExternalInput")
    iota_d = nc.dram_tensor("iota", [128, 128], f32, kind="ExternalInput")
    ones_d = nc.dram_tensor("onesrow", [1, 128], f32, kind="ExternalInput")
    w1l_d = nc.dram_tensor("w1l", [f_in, F1], f32, kind="ExternalInput")
    w1r_d = nc.dram_tensor("w1r", [f_in, F1], f32, kind="ExternalInput")
    b1l_d = nc.dram_tensor("b1l", [1, F1], f32, kind="ExternalInput")
    w2l_d = nc.dram_tensor("w2l", [F1, 64], f32, kind="ExternalInput")  # padded 32->64
    w2r_d = nc.dram_tensor("w2r", [F1, F2], f32, kind="ExternalInput")
    b2l_d = nc.dram_tensor("b2l", [1, F2], f32, kind="ExternalInput")
    gpool_d = nc.dram_tensor("gpool", [128, nblk * N_GRAPHS], f32, kind="ExternalInput")
    invg_d = nc.dram_tensor("invg", [N_GRAPHS, 1], f32, kind="ExternalInput")
    wl1_d = nc.dram_tensor("wl1", [F2, F2], f32, kind="ExternalInput")
    bl1_d = nc.dram_tensor("bl1", [1, F2], f32, kind="ExternalInput")
    wl2_d = nc.dram_tensor("wl2", [F2, LAT], f32, kind="ExternalInput")
    bl2_d = nc.dram_tensor("bl2", [1, LAT], f32, kind="ExternalInput")
    wd1_d = nc.dram_tensor("wd1", [LAT, F2], f32, kind="ExternalInput")
    bd1_d = nc.dram_tensor("bd1", [1, F2], f32, kind="ExternalInput")
    wd2_d = nc.dram_tensor("wd2", [F2, F2], f32, kind="ExternalInput")
    bd2_d = nc.dram_tensor("bd2", [1, F2], f32, kind="ExternalInput")
    wd3_d = nc.dram_tensor("wd3", [F2, FOUT], f32, kind="ExternalInput")
    bd3_d = nc.dram_tensor("bd3", [1, FOUT], f32, kind="ExternalInput")

    xl_chunk = nc.dram_tensor("xl_chunk", [npad, F1], f32, kind="Internal")
    xl_full = nc.dram_tensor("xl_full", [NCORES * npad, F1], f32, kind="Internal", addr_space="Shared")
    h1l_chunk = nc.dram_tensor("h1l_chunk", [npad, 64], f32, kind="Internal")
    h1l_full = nc.dram_tensor("h1l_full", [NCORES * npad, 64], f32, kind="Internal", addr_space="Shared")
    gs_part = nc.dram_tensor("gs_part", [N_GRAPHS, F2], f32, kind="Internal")
    gs_full = nc.dram_tensor("gs_full", [N_GRAPHS, F2], f32, kind="Internal", addr_space="Shared")

    enc_out = nc.dram_tensor("enc_t", [LAT, N_GRAPHS], f32, kind="ExternalOutput")
    rec_out = nc.dram_tensor("rec_t", [FOUT, N_GRAPHS], f32, kind="ExternalOutput")

    nchunks = -(-nblk // CHB)

    with tile.TileContext(nc) as tc:
        with (
            tc.tile_pool(name="constp", bufs=1) as constp,
            tc.tile_pool(name="resp", bufs=1) as resp,
            tc.tile_pool(name="workp", bufs=3) as workp,
            tc.tile_pool(name="msgp", bufs=2) as msgp,
            tc.tile_pool(name="spool", bufs=6) as spool,
            tc.tile_pool(name="psump", bufs=2, space="PSUM") as psump,
        ):
            # ---------------- constants ----------------
            iota = constp.tile([128, 128], f32)
            nc.sync.dma_start(out=iota[:], in_=iota_d[:])
            ident = constp.tile([128, 128], f32)
            make_identity(nc, ident[:])
            ones = constp.tile([1, 128], f32)
            nc.sync.dma_start(out=ones[:], in_=ones_d[:])
            w1l = [constp.tile([128, F1], f32, tag=f"w1l{k}", name=f"w1l{k}") for k in range(kin)]
            w1r = [constp.tile([128, F1], f32, tag=f"w1r{k}", name=f"w1r{k}") for k in range(kin)]
            for k in range(kin):
                nc.sync.dma_start(out=w1l[k][:], in_=w1l_d[128 * k:128 * (k + 1), :])
                nc.sync.dma_start(out=w1r[k][:], in_=w1r_d[128 * k:128 * (k + 1), :])
            b1l = constp.tile([1, F1], f32)
            nc.sync.dma_start(out=b1l[:], in_=b1l_d[:])
            w2l = constp.tile([F1, 64], f32)
            nc.sync.dma_start(out=w2l[:], in_=w2l_d[:])
            w2r = constp.tile([F1, F2], f32)
            nc.sync.dma_start(out=w2r[:], in_=w2r_d[:])
            b2l = constp.tile([1, F2], f32)
            nc.sync.dma_start(out=b2l[:], in_=b2l_d[:])
            dstrel = constp.tile([128, 2 * nblk * t_blk], f32)
            nc.sync.dma_start(out=dstrel[:], in_=dstrel_d[:])
            invc = constp.tile([128, 2 * nblk * t_blk], f32)
            nc.sync.dma_start(out=invc[:], in_=invc_d[:])
            idxs = [constp.tile([128, nidxh // 16], i16, tag=f"idx{h}", name=f"idxs{h}") for h in (0, 1)]
            for h in (0, 1):
                nc.sync.dma_start(out=idxs[h][:], in_=idx_d[h][:])
            gpool = constp.tile([128, nblk * N_GRAPHS], f32)
            nc.sync.dma_start(out=gpool[:], in_=gpool_d[:])
            invg = constp.tile([N_GRAPHS, 1], f32)
            nc.sync.dma_start(out=invg[:], in_=invg_d[:])
            hw_ = {}
            for nm, d_ in [("wl1", wl1_d), ("bl1", bl1_d), ("wl2", wl2_d), ("bl2", bl2_d),
                           ("wd1", wd1_d), ("bd1", bd1_d), ("wd2", wd2_d), ("bd2", bd2_d),
                           ("wd3", wd3_d), ("bd3", bd3_d)]:
                t_ = constp.tile(list(d_.shape), f32, tag=nm, name=nm)
                nc.sync.dma_start(out=t_[:], in_=d_[:])
                hw_[nm] = t_

            # resident activations
            xt = [resp.tile([128, npad], f32, tag=f"xt{k}", name=f"xt{k}") for k in range(kin)]
            for k in range(kin):
                nc.sync.dma_start(out=xt[k][:], in_=xt_d[128 * k:128 * (k + 1), :])
            h1t = resp.tile([128, npad], f32)  # h1 transposed [feat, node]

            # ---------------- phase 1: xl table = x @ W1l  (node-major) ----------------
            xl_stores = []
            for b in range(nblk):
                bs = b * BLK
                ps = psump.tile([128, F1], f32, tag="mm")
                for k in range(kin):
                    nc.tensor.matmul(out=ps[:], lhsT=xt[k][:, bs:bs + BLK], rhs=w1l[k][:],
                                     start=(k == 0), stop=(k == kin - 1))
                sb = workp.tile([128, F1], f32, tag="xl_sb")
                nc.vector.tensor_copy(out=sb[:], in_=ps[:])
                st = nc.sync.dma_start(out=xl_chunk[bs:bs + BLK, :], in_=sb[:])
                xl_stores.append(st)
            ag1 = nc.gpsimd.collective_compute(
                "AllGather", ALU.bypass, replica_groups=rg, ins=[xl_chunk[:]], outs=[xl_full[:]])
            for st in xl_stores:
                add_dep_helper(ag1.ins, st.ins, True, "ag1 after xl stores")

            # ---------------- shared: one conv layer's aggregation ----------------
            def s_tile_for(h, b, t):
                col = (h * nblk + b) * t_blk + t
                s = spool.tile([128, 128], f32, tag="s", name="s")
                nc.vector.tensor_scalar(
                    out=s[:], in0=iota[:],
                    scalar1=dstrel[:, col:col + 1], scalar2=invc[:, col:col + 1],
                    op0=ALU.is_equal, op1=ALU.mult)
                return s

            def gather_chunk(c, table, elem, msg_tag, dep, layer):
                b0 = c * CHB
                nb = min(CHB, nblk - b0)
                ms = {}
                for h in (0, 1):
                    m = msgp.tile([128, CHB * t_blk, elem], f32, tag=msg_tag, name=f"m{layer}_{c}_{h}")
                    nid = nb * t_blk * BLK
                    g = nc.gpsimd.dma_gather(
                        out_ap=m[:, :nb * t_blk, :],
                        in_ap=table[h * half:(h + 1) * half, :],
                        idxs_ap=idxs[h][:, b0 * t_blk * 8: b0 * t_blk * 8 + nid // 16],
                        num_idxs=nid, num_idxs_reg=nid, elem_size=elem,
                        single_packet=False)
                    add_dep_helper(g.ins, dep.ins, True, "gather after allgather")
                    ms[h] = m
                return ms

            # ---------------- phase 2: L1 ----------------
            for c in range(nchunks):
                ms = gather_chunk(c, xl_full, F1, "msg1", ag1, 1)
                for b in range(c * CHB, min(nblk, (c + 1) * CHB)):
                    bs = b * BLK
                    ps = psump.tile([128, F1], f32, tag="mm", name="ps1")
                    for k in range(kin):
                        nc.tensor.matmul(out=ps[:], lhsT=xt[k][:, bs:bs + BLK], rhs=w1r[k][:],
                                         start=(k == 0), stop=False)
                    nc.tensor.matmul(out=ps[:], lhsT=ones[:], rhs=b1l[:], start=False, stop=False)
                    for h in (0, 1):
                        for t in range(t_blk):
                            s = s_tile_for(h, b, t)
                            pos = (b - c * CHB) * t_blk + t
                            nc.tensor.matmul(out=ps[:], lhsT=s[:], rhs=ms[h][:, pos, :],
                                             start=False, stop=(h == 1 and t == t_blk - 1))
                    h1b = workp.tile([128, F1], f32, tag="h1b", name="h1b")
                    nc.scalar.activation(out=h1b[:], in_=ps[:], func=AF.Relu)
                    pst = psump.tile([128, 128], f32, tag="tr", name="pst")
                    nc.tensor.transpose(out=pst[:], in_=h1b[:], identity=ident[:])
                    nc.vector.tensor_copy(out=h1t[:, bs:bs + BLK], in_=pst[:])

            # ---------------- phase 3: h1l table = h1 @ W2l ----------------
            h1l_stores = []
            for b in range(nblk):
                bs = b * BLK
                ps = psump.tile([128, 64], f32, tag="mm", name="ps3")
                nc.tensor.matmul(out=ps[:], lhsT=h1t[:, bs:bs + BLK], rhs=w2l[:], start=True, stop=True)
                sb = workp.tile([128, 64], f32, tag="h1l_sb")
                nc.vector.tensor_copy(out=sb[:], in_=ps[:])
                st = nc.sync.dma_start(out=h1l_chunk[bs:bs + BLK, :], in_=sb[:])
                h1l_stores.append(st)
            ag2 = nc.gpsimd.collective_compute(
                "AllGather", ALU.bypass, replica_groups=rg, ins=[h1l_chunk[:]], outs=[h1l_full[:]])
            for st in h1l_stores:
                add_dep_helper(ag2.ins, st.ins, True, "ag2 after h1l stores")

            # ---------------- phase 4: L2 + pooling ----------------
            psg = psump.tile([N_GRAPHS, F2], f32, tag="g", name="psg")
            for c in range(nchunks):
                ms = gather_chunk(c, h1l_full, 64, "msg2", ag2, 2)
                for b in range(c * CHB, min(nblk, (c + 1) * CHB)):
                    bs = b * BLK
                    ps = psump.tile([128, F2], f32, tag="mm", name="ps4")
                    nc.tensor.matmul(out=ps[:], lhsT=h1t[:, bs:bs + BLK], rhs=w2r[:], start=True, stop=False)
                    nc.tensor.matmul(out=ps[:], lhsT=ones[:], rhs=b2l[:], start=False, stop=False)
                    for h in (0, 1):
                        for t in range(t_blk):
                            s = s_tile_for(h, b, t)
                            pos = (b - c * CHB) * t_blk + t
                            nc.tensor.matmul(out=ps[:], lhsT=s[:], rhs=ms[h][:, pos, 0:F2],
                                             start=False, stop=(h == 1 and t == t_blk - 1))
                    h2b = workp.tile([128, F2], f32, tag="h2b", name="h2b")
                    nc.scalar.activation(out=h2b[:], in_=ps[:], func=AF.Relu)
                    nc.tensor.matmul(out=psg[:], lhsT=gpool[:, b * N_GRAPHS:(b + 1) * N_GRAPHS],
                                     rhs=h2b[:], start=(b == 0), stop=(b == nblk - 1))
            gsb = workp.tile([N_GRAPHS, F2], f32, tag="gsb")
            nc.vector.tensor_copy(out=gsb[:], in_=psg[:])
            stg = nc.sync.dma_start(out=gs_part[:], in_=gsb[:])
            ar = nc.gpsimd.collective_compute(
                "AllReduce", ALU.add, replica_groups=rg, ins=[gs_part[:]], outs=[gs_full[:]])
            add_dep_helper(ar.ins, stg.ins, True, "ar after gsum store")

            # ---------------- phase 5: head (replicated) ----------------
            gnm = workp.tile([N_GRAPHS, F2], f32, tag="gnm")
            ld = nc.sync.dma_start(out=gnm[:], in_=gs_full[:])
            add_dep_helper(ld.ins, ar.ins, True, "g load after allreduce")
            nc.vector.tensor_scalar_mul(gnm[:], gnm[:], invg[:, 0:1])
            pst = psump.tile([F2, N_GRAPHS], f32, tag="tr", name="pst_h")
            nc.tensor.transpose(out=pst[:], in_=gnm[:], identity=ident[0:N_GRAPHS, 0:N_GRAPHS])
            gt = workp.tile([F2, N_GRAPHS], f32, tag="gt")
            nc.vector.tensor_copy(out=gt[:], in_=pst[:])

            def head_stage(w_t, b_t, rhs_t, p_out, act, tag):
                ps = psump.tile([p_out, N_GRAPHS], f32, tag="mm", name=f"psh_{tag}")
                nc.tensor.matmul(out=ps[:], lhsT=w_t[:], rhs=rhs_t[:], start=True, stop=False)
                nc.tensor.matmul(out=ps[:], lhsT=b_t[:], rhs=ones[0:1, 0:N_GRAPHS], start=False, stop=True)
                o = workp.tile([p_out, N_GRAPHS], f32, tag=f"hd_{tag}", name=f"hd_{tag}")
                if act == "relu":
                    nc.scalar.activation(out=o[:], in_=ps[:], func=AF.Relu)
                elif act == "leaky":
                    tmp = workp.tile([p_out, N_GRAPHS], f32, tag=f"hdt_{tag}", name=f"hdt_{tag}")
                    nc.vector.tensor_scalar_mul(tmp[:], ps[:], 0.1)
                    nc.vector.tensor_tensor(out=o[:], in0=ps[:], in1=tmp[:], op=ALU.max)
                else:
                    nc.vector.tensor_copy(out=o[:], in_=ps[:])
                return o

            z1 = head_stage(hw_["wl1"], hw_["bl1"], gt, F2, "relu", "z1")
            enc = head_stage(hw_["wl2"], hw_["bl2"], z1, LAT, "leaky", "enc")
            z2 = head_stage(hw_["wd1"], hw_["bd1"], enc, F2, "leaky", "z2")
            z3 = head_stage(hw_["wd2"], hw_["bd2"], z2, F2, "leaky", "z3")
            rec = head_stage(hw_["wd3"], hw_["bd3"], z3, FOUT, "none", "rec")
            nc.sync.dma_start(out=enc_out[:], in_=enc[:])
            nc.sync.dma_start(out=rec_out[:], in_=rec[:])

    nc.compile()
    return nc


# ---------------- host-side packing ----------------

def _wrap_idx(idx):
    n = idx.shape[0]
    w = idx.reshape(n // 16, 16).T
    return np.tile(w, (8, 1)).astype(np.int16)


def pack_inputs(x, edge_index, batch, params, n_nodes, f_in):
    """Returns (t_blk, in_maps list of per-core dicts)."""
    npc, nblk, npad, half, kin = _cfg(n_nodes, f_in)
    x = np.asarray(x, np.float32)
    ei = np.asarray(edge_index).astype(np.int64)
    batch = np.asarray(batch).astype(np.int64)
    src, dst = ei[0], ei[1]

    cnt = np.bincount(dst, minlength=n_nodes).astype(np.float32)
    invc_node = (1.0 / np.maximum(cnt, 1.0)).astype(np.float32)
    gcnt = np.bincount(batch, minlength=N_GRAPHS).astype(np.float32)
    invg = (1.0 / np.maximum(gcnt, 1.0)).astype(np.float32).reshape(N_GRAPHS, 1)

    srow = (src // npc) * npad + (src % npc)  # padded table row of each source
    dcore = dst // npc

    # per (core, half, block) edge grouping
    per_core = []
    t_need = 1
    for c in range(NCORES):
        sel = dcore == c
        s_r = srow[sel]
        d_l = dst[sel] - c * npc
        iv = invc_node[dst[sel]]
        hh = (s_r >= half).astype(np.int64)
        bb = d_l // BLK
        key = hh * nblk + bb
        order = np.argsort(key, kind="stable")
        s_r, d_l, iv, key = s_r[order], d_l[order], iv[order], key[order]
        bounds = np.searchsorted(key, np.arange(2 * nblk + 1))
        cnts = np.diff(bounds)
        t_need = max(t_need, int(np.max(-(-cnts // BLK))) if cnts.size else 1)
        per_core.append((s_r, d_l, iv, bounds))

    t_blk = max(t_need, 1)
    nidxh = nblk * t_blk * BLK

    iota_np = np.tile(np.arange(128, dtype=np.float32), (128, 1))
    ones_np = np.ones((1, 128), np.float32)
    w2l_pad = np.zeros((F1, 64), np.float32)
    w2l_pad[:, :F2] = np.asarray(params["W2l"], np.float32)

    shared = {
        "iota": iota_np, "onesrow": ones_np,
        "w1l": np.asarray(params["W1l"], np.float32),
        "w1r": np.asarray(params["W1r"], np.float32),
        "b1l": np.asarray(params["b1l"], np.float32).reshape(1, F1),
        "w2l": w2l_pad,
        "w2r": np.asarray(params["W2r"], np.float32),
        "b2l": np.asarray(params["b2l"], np.float32).reshape(1, F2),
        "invg": invg,
        "wl1": np.asarray(params["Wl1"], np.float32),
        "bl1": np.asarray(params["bl1"], np.float32).reshape(1, F2),
        "wl2": np.asarray(params["Wl2"], np.float32),
        "bl2": np.asarray(params["bl2"], np.float32).reshape(1, LAT),
        "wd1": np.asarray(params["Wd1"], np.float32),
        "bd1": np.asarray(params["bd1"], np.float32).reshape(1, F2),
        "wd2": np.asarray(params["Wd2"], np.float32),
        "bd2": np.asarray(params["bd2"], np.float32).reshape(1, F2),
        "wd3": np.asarray(params["Wd3"], np.float32),
        "bd3": np.asarray(params["bd3"], np.float32).reshape(1, FOUT),
    }

    in_maps = []
    for c in range(NCORES):
        s_r, d_l, iv, bounds = per_core[c]
        idx_arr = np.zeros((2, nidxh), np.int64)
        dr_arr = np.full((2, nblk, t_blk * BLK), 999.0, np.float32)
        iv_arr = np.zeros((2, nblk, t_blk * BLK), np.float32)
        for h in (0, 1):
            for b in range(nblk):
                g0, g1 = bounds[h * nblk + b], bounds[h * nblk + b + 1]
                n = g1 - g0
                p0 = b * t_blk * BLK
                idx_arr[h, p0:p0 + n] = s_r[g0:g1] - h * half
                dr_arr[h, b, :n] = d_l[g0:g1] - b * BLK
                iv_arr[h, b, :n] = iv[g0:g1]
        assert idx_arr.max() < half and idx_arr.min() >= 0

        xt_c = np.zeros((f_in, npad), np.float32)
        xt_c[:, :npc] = x[c * npc:(c + 1) * npc].T

        gpool_c = np.zeros((128, nblk * N_GRAPHS), np.float32)
        bvals = batch[c * npc:(c + 1) * npc]
        node_pos = np.arange(npc)
        gpool_c[node_pos % BLK, (node_pos // BLK) * N_GRAPHS + bvals] = 1.0

        def tilecols(a):  # [nblk, t_blk*128] halves -> [128, 2*nblk*t_blk]
            return np.concatenate(
                [a[h, b].reshape(t_blk, BLK).T for h in (0, 1) for b in range(nblk)],
                axis=1).astype(np.float32)

        m = dict(shared)
        m["xt"] = xt_c
        m["idx0"] = _wrap_idx(idx_arr[0])
        m["idx1"] = _wrap_idx(idx_arr[1])
        m["dstrel"] = tilecols(dr_arr)
        m["invc"] = tilecols(iv_arr)
        m["gpool"] = gpool_c
        in_maps.append(m)
    return t_blk, in_maps


def run_device(n_nodes, f_in, t_blk, in_maps, trace=False):
    from concourse.bass_utils import run_bass_kernel_spmd
    key = (n_nodes, f_in, t_blk)
    if key not in _PROG_CACHE:
        _PROG_CACHE[key] = build_program(n_nodes, f_in, t_blk)
    nc = _PROG_CACHE[key]
    last_exc = None
    for attempt in range(3):
        try:
            res = run_bass_kernel_spmd(nc, in_maps, core_ids=list(range(NCORES)), trace=trace)
            return res
        except Exception as e:  # transient NRT device errors happen; retry
            last_exc = e
            if "UNRECOVERABLE" not in str(e) and "UNAVAILABLE" not in str(e):
                raise
    raise last_exc


def time_device(n_nodes, f_in, t_blk, in_maps, iters=5):
    """Steady-state device wall-clock per execution (ns), with inputs resident
    on device. Measures dispatch + NEFF execution (no host transfer)."""
    import time
    import jax
    import numpy as np_
    from jax.sharding import Mesh, PartitionSpec, NamedSharding
    from jax.experimental.shard_map import shard_map
    from concourse import bass2jax, mybir

    key = (n_nodes, f_in, t_blk)
    nc = _PROG_CACHE[key]
    bass2jax.install_neuronx_cc_hook()

    n_cores = NCORES
    in_names, out_names, out_avals, zero_outs = [], [], [], []
    partition_name = nc.partition_id_tensor.name if nc.partition_id_tensor else None
    for alloc in nc.m.functions[0].allocations:
        if not isinstance(alloc, mybir.MemoryLocationSet):
            continue
        name = alloc.memorylocations[0].name
        if alloc.kind == "ExternalInput":
            if name != partition_name:
                in_names.append(name)
        elif alloc.kind == "ExternalOutput":
            out_names.append(name)
            shape = tuple(alloc.tensor_shape)
            dtype = mybir.dt.np(alloc.dtype)
            out_avals.append(jax.core.ShapedArray(shape, dtype))
            zero_outs.append(np_.zeros(shape, dtype))
    n_params = len(in_names)
    all_in_names = in_names + out_names
    if partition_name is not None:
        all_in_names.append(partition_name)

    def _body(*args):
        operands = list(args)
        if partition_name is not None:
            operands.append(bass2jax.partition_id_tensor())
        outs = bass2jax._bass_exec_p.bind(
            *operands, out_avals=tuple(out_avals), in_names=tuple(all_in_names),
            out_names=tuple(out_names), lowering_input_output_aliases=(),
            sim_require_finite=True, sim_require_nnan=True, nc=nc)
        return tuple(outs)

    devices = jax.devices()[:n_cores]
    mesh = Mesh(np_.asarray(devices), ("core",))
    donate = tuple(range(n_params, n_params + len(out_names)))
    sharded = jax.jit(
        shard_map(_body, mesh=mesh,
                  in_specs=(PartitionSpec("core"),) * (n_params + len(out_names)),
                  out_specs=(PartitionSpec("core"),) * len(out_names), check_rep=False),
        donate_argnums=donate, keep_unused=True)

    sh = NamedSharding(mesh, PartitionSpec("core"))
    concat_in = [
        jax.device_put(
            np_.concatenate([np_.asarray(in_maps[c][nm]) for c in range(n_cores)], axis=0), sh)
        for nm in in_names
    ]
    times = []
    for it in range(iters + 1):
        czeros = [jax.device_put(
            np_.zeros((n_cores * z.shape[0], *z.shape[1:]), z.dtype), sh) for z in zero_outs]
        jax.block_until_ready(czeros)
        t0 = time.perf_counter()
        outs = sharded(*concat_in, *czeros)
        jax.block_until_ready(outs)
        dt_ = time.perf_counter() - t0
        if it > 0:  # first call compiles/warms
            times.append(dt_)
    return min(times) * 1e9, [t * 1e9 for t in times]


def kernel(x, edge_index, batch, W1l, b1l, W1r, W2l, b2l, W2r,
           Wl1, bl1, Wl2, bl2, Wd1, bd1, Wd2, bd2, Wd3, bd3):
    params = dict(W1l=W1l, b1l=b1l, W1r=W1r, W2l=W2l, b2l=b2l, W2r=W2r,
                  Wl1=Wl1, bl1=bl1, Wl2=Wl2, bl2=bl2, Wd1=Wd1, bd1=bd1,
                  Wd2=Wd2, bd2=bd2, Wd3=Wd3, bd3=bd3)
    t_blk, in_maps = pack_inputs(x, edge_index, batch, params, N_NODES, F_IN)
    trace = os.environ.get("GNN_TRACE") == "1"
    res = run_device(N_NODES, F_IN, t_blk, in_maps, trace=trace)
    kernel.last_results = res
    kernel.last_tblk = t_blk
    kernel.last_in_maps = in_maps
    enc = np.ascontiguousarray(res.results[0]["enc_t"].T)
    rec = np.ascontiguousarray(res.results[0]["rec_t"].T)
    return enc, rec
